# revision 6
# baseline (speedup 1.0000x reference)
"""AFNO2D layer on 8 Trainium2 NeuronCores.

Sharding: channel-block parallel. C=768 = 8 blocks x 96 channels; the complex
MLP is block-diagonal over exactly these blocks, and the 2D FFT is independent
per channel — so core i handles channel block i end-to-end with zero
collectives.

Per-core pipeline (per batch b, all DFTs as dense matmuls on the 128x128 PE):
  S1  W-axis rfft, Hermitian-packed:  lhsT=Fpack[w,128] -> Ypack[fpack,(c,h)]
  TA  PE-transpose corner turn    -> Yt[h,(f,c)]
  S2  H-axis complex FFT (2 real matmuls + DVE combines) -> Zr,Zi [g,(f,c)]
  TB  PE-transpose                -> Zrt,Zit [c,(f,g)]
  L1  complex 96x96 matmul + bias + ReLU (PSUM accumulate pairs)
  L2  complex 96x96 matmul + bias; softshrink on DVE
  TC  PE-transpose                -> Or,Oi [g,(f,c)]
  S5  inverse H FFT + combines, packed -> ZIboth [h,(c,fpack)]
  TD  PE-transpose                -> ZIpack [fpack,(h,c)]
  S6  inverse W rfft (packed lhsT=Apack) + residual add -> out
Matmul dtype bf16 (PSUM accumulates f32); residual path f32. The output is
x + delta with ||delta||/||x|| ~ 0.05, so bf16 error in delta is attenuated
~20x in the final rel-err.
"""

import sys

import numpy as np

try:
    import concourse  # noqa: F401
except ImportError:
    sys.path.insert(0, "/opt/trn_rl_repo")

import ml_dtypes

import concourse.bass as bass
import concourse.bacc as bacc
import concourse.mybir as mybir
import concourse.tile as tile
from concourse.bass_utils import run_bass_kernel_spmd

BF16 = ml_dtypes.bfloat16
DT = mybir.dt

B = 4
H = 128
W = 128
C = 96  # per-core channels (one MLP block)
F = 65  # rfft freqs along W
LAM = 0.01
N_CORES = 8

_CACHE = {}


def _host_matrices():
    """DFT matrices, all as matmul lhsT ([K, M]) layouts, bf16."""
    I = np.eye(W)
    R = np.fft.rfft(I, axis=1, norm="ortho")  # [w, f]: Y = x @ R
    fpack = np.concatenate([R.real, R.imag[:, 1:64]], axis=1)  # [w, 128]
    Dm = np.fft.fft(np.eye(H), axis=1, norm="ortho")  # [h, g]: Z = Y @ Dm
    DmI = np.fft.ifft(np.eye(H), axis=1, norm="ortho")  # [g, h]
    Ar = np.zeros((F, W))
    Ai = np.zeros((F, W))
    for f in range(F):
        e = np.zeros(F, dtype=complex)
        e[f] = 1.0
        Ar[f] = np.fft.irfft(e, n=W, norm="ortho")
        e = np.zeros(F, dtype=complex)
        e[f] = 1j
        Ai[f] = np.fft.irfft(e, n=W, norm="ortho")
    apack = np.concatenate([Ar, Ai[1:64]], axis=0)  # [fpack, w]
    c = lambda a: np.ascontiguousarray(a.astype(BF16))
    return {
        "fpack": c(fpack),
        "drt": c(Dm.real),
        "dit": c(Dm.imag),
        "ditn": c(-Dm.imag),
        "dirt": c(DmI.real),
        "diit": c(DmI.imag),
        "diitn": c(-DmI.imag),
        "apack": c(apack),
        "ident": c(np.eye(128)),
    }


def _build_nc():
    nc = bacc.Bacc(
        "TRN2", target_bir_lowering=False, debug=False, num_devices=N_CORES
    )

    def din(name, shape, dt):
        return nc.dram_tensor(name, shape, dt, kind="ExternalInput")

    x16 = din("x16", [B, W, C, H], DT.bfloat16)
    xres = din("xres", [B, H, W, C], DT.float32)
    mats = {
        k: din(k, [128, 128], DT.bfloat16)
        for k in ["fpack", "drt", "dit", "ditn", "dirt", "diit", "diitn", "apack", "ident"]
    }
    wts = {k: din(k, [C, C], DT.bfloat16)
           for k in ["w1r", "w1i", "w1in", "w2r", "w2i", "w2in"]}
    bs = {k: din(k, [C, 1], DT.float32) for k in ["b1r", "b1i", "b2r", "b2i"]}
    out_ext = nc.dram_tensor("out", [B, H, W, C], DT.float32, kind="ExternalOutput")

    CH = C * H          # 12288
    FC = F * C          # 6240
    FG = F * 128        # 8320

    with tile.TileContext(nc) as tc:
        with (
            tc.tile_pool(name="const", bufs=1) as cpool,
            tc.tile_pool(name="big", bufs=3) as bpool,
            tc.tile_pool(name="pa", bufs=3) as papool,
            tc.tile_pool(name="pb", bufs=4) as pbpool,
            tc.tile_pool(name="sm", bufs=2) as spool,
            tc.tile_pool(name="io", bufs=2) as iopool,
            tc.tile_pool(name="ps", bufs=8, space="PSUM") as pspool,
        ):
            M = {}
            for k in mats:
                M[k] = cpool.tile([128, 128], DT.bfloat16, tag=f"m_{k}", name=f"m_{k}")
                nc.sync.dma_start(M[k][:], mats[k][:])
            Wt = {}
            for k in wts:
                Wt[k] = cpool.tile([C, C], DT.bfloat16, tag=f"w_{k}", name=f"wt_{k}")
                nc.sync.dma_start(Wt[k][:], wts[k][:])
            Bt = {}
            for k in bs:
                Bt[k] = cpool.tile([C, 1], DT.float32, tag=f"b_{k}", name=f"bt_{k}")
                nc.sync.dma_start(Bt[k][:], bs[k][:])
            ID = M["ident"]

            for b in range(B):
                # ---- load x (pre-packed bf16 [w, (c, h)]) ----
                xb = bpool.tile([128, CH], DT.bfloat16, tag="big")
                nc.sync.dma_start(xb[:], x16[b].rearrange("w c h -> w (c h)"))

                # ---- S1: W-rfft packed ----
                yp = bpool.tile([128, CH], DT.bfloat16, tag="big")
                for k in range(CH // 512):
                    s = slice(512 * k, 512 * (k + 1))
                    ps = pspool.tile([128, 512], DT.float32, tag="ps")
                    nc.tensor.matmul(ps[:], M["fpack"][:], xb[:, s],
                                     start=True, stop=True)
                    nc.scalar.copy(yp[:, s], ps[:])

                # ---- TA: [fpack,(c,h)] -> Yt [h,(f,c)] ----
                yt = bpool.tile([128, CH], DT.bfloat16, tag="big")
                ypv = yp[:].rearrange("p (c h) -> p c h", h=H)
                ytv = yt[:].rearrange("p (f c) -> p c f", c=C)
                for c0 in range(0, C, 4):
                    psT = pspool.tile([128, 512], DT.bfloat16, tag="ps")
                    for j in range(4):
                        nc.tensor.transpose(
                            psT[:, 128 * j:128 * (j + 1)], ypv[:, c0 + j, :], ID[:]
                        )
                    nc.scalar.copy(ytv[:, c0:c0 + 4, :], psT[:])

                # ---- S2: H-axis complex FFT (PE-accumulated combines) ----
                zr = papool.tile([128, FC], DT.bfloat16, tag="pa")
                zi = papool.tile([128, FC], DT.bfloat16, tag="pa")
                # edges f=0 (cols 0:96) and f=64 (cols 6144:6240): Yi=0 there
                for cols in [slice(0, 96), slice(6144, 6240)]:
                    pe1 = pspool.tile([128, 96], DT.float32, tag="ps")
                    pe2 = pspool.tile([128, 96], DT.float32, tag="ps")
                    nc.tensor.matmul(pe1[:], M["drt"][:], yt[:, cols],
                                     start=True, stop=True)
                    nc.tensor.matmul(pe2[:], M["dit"][:], yt[:, cols],
                                     start=True, stop=True)
                    nc.scalar.copy(zr[:, cols], pe1[:])
                    nc.scalar.copy(zi[:, cols], pe2[:])
                for j in range(12):
                    sa = slice(96 + 504 * j, 96 + 504 * (j + 1))      # fr f=1..63
                    sb = slice(6240 + 504 * j, 6240 + 504 * (j + 1))  # fi f=1..63
                    pszr = pspool.tile([128, 504], DT.float32, tag="ps")
                    pszi = pspool.tile([128, 504], DT.float32, tag="ps")
                    nc.tensor.matmul(pszr[:], M["drt"][:], yt[:, sa], start=True, stop=False)
                    nc.tensor.matmul(pszr[:], M["ditn"][:], yt[:, sb], start=False, stop=True)
                    nc.tensor.matmul(pszi[:], M["drt"][:], yt[:, sb], start=True, stop=False)
                    nc.tensor.matmul(pszi[:], M["dit"][:], yt[:, sa], start=False, stop=True)
                    nc.vector.tensor_copy(zr[:, sa], pszr[:])
                    nc.vector.tensor_copy(zi[:, sa], pszi[:])

                # ---- TB: [g,(f,c)] -> [c,(f,g)] ----
                zrt = pbpool.tile([C, FG], DT.bfloat16, tag="pb")
                zit = pbpool.tile([C, FG], DT.bfloat16, tag="pb")
                for (src, dst) in [(zr, zrt), (zi, zit)]:
                    sv = src[:].rearrange("p (f c) -> p f c", c=C)
                    for f0 in range(0, 64, 4):
                        psB = pspool.tile([C, 512], DT.bfloat16, tag="ps")
                        for j in range(4):
                            nc.tensor.transpose(
                                psB[:, 128 * j:128 * (j + 1)], sv[:, f0 + j, :], ID[:]
                            )
                        nc.scalar.copy(dst[:, 128 * f0:128 * (f0 + 4)], psB[:])
                    psB = pspool.tile([C, 128], DT.bfloat16, tag="ps")
                    nc.tensor.transpose(psB[:], sv[:, 64, :], ID[:])
                    nc.scalar.copy(dst[:, 128 * 64:128 * 65], psB[:])

                # ---- L1 + L2 MLP fused (chunks over (f,g)) ----
                o2r = pbpool.tile([C, FG], DT.bfloat16, tag="pb")
                o2i = pbpool.tile([C, FG], DT.bfloat16, tag="pb")
                chunks = [slice(512 * k, min(512 * (k + 1), FG))
                          for k in range((FG + 511) // 512)]
                for s in chunks:
                    n = s.stop - s.start
                    psr = pspool.tile([C, n], DT.float32, tag="ps")
                    psi = pspool.tile([C, n], DT.float32, tag="ps")
                    nc.tensor.matmul(psr[:], Wt["w1r"][:], zrt[:, s], start=True, stop=False)
                    nc.tensor.matmul(psr[:], Wt["w1in"][:], zit[:, s], start=False, stop=True)
                    nc.tensor.matmul(psi[:], Wt["w1r"][:], zit[:, s], start=True, stop=False)
                    nc.tensor.matmul(psi[:], Wt["w1i"][:], zrt[:, s], start=False, stop=True)
                    o1rc = spool.tile([C, 512], DT.bfloat16, tag="o1r", name="o1rc")
                    o1ic = spool.tile([C, 512], DT.bfloat16, tag="o1i", name="o1ic")
                    nc.scalar.activation(o1rc[:, :n], psr[:],
                                         mybir.ActivationFunctionType.Relu,
                                         bias=Bt["b1r"][:, 0:1])
                    nc.scalar.activation(o1ic[:, :n], psi[:],
                                         mybir.ActivationFunctionType.Relu,
                                         bias=Bt["b1i"][:, 0:1])
                    psr2 = pspool.tile([C, n], DT.float32, tag="ps")
                    psi2 = pspool.tile([C, n], DT.float32, tag="ps")
                    nc.tensor.matmul(psr2[:], Wt["w2r"][:], o1rc[:, :n], start=True, stop=False)
                    nc.tensor.matmul(psr2[:], Wt["w2in"][:], o1ic[:, :n], start=False, stop=True)
                    nc.tensor.matmul(psi2[:], Wt["w2r"][:], o1ic[:, :n], start=True, stop=False)
                    nc.tensor.matmul(psi2[:], Wt["w2i"][:], o1rc[:, :n], start=False, stop=True)
                    for ps, bias, dst in [(psr2, "b2r", o2r), (psi2, "b2i", o2i)]:
                        t = spool.tile([C, 512], DT.bfloat16, tag="ss1", name="sst")
                        tcl = spool.tile([C, 512], DT.bfloat16, tag="ss2", name="sscl")
                        nc.scalar.activation(t[:, :n], ps[:],
                                             mybir.ActivationFunctionType.Identity,
                                             bias=Bt[bias][:, 0:1])
                        nc.vector.tensor_scalar(
                            tcl[:, :n], t[:, :n], -LAM, LAM,
                            mybir.AluOpType.max, mybir.AluOpType.min)
                        nc.vector.tensor_sub(dst[:, s], t[:, :n], tcl[:, :n])

                # ---- TC: [c,(f,g)] -> [g,(f,c)] ----
                or_ = papool.tile([128, FC], DT.bfloat16, tag="pa")
                oi_ = papool.tile([128, FC], DT.bfloat16, tag="pa")
                for (src, dst) in [(o2r, or_), (o2i, oi_)]:
                    sv = src[:].rearrange("p (f g) -> p f g", g=128)
                    for f0 in range(0, F, 5):
                        psC = pspool.tile([128, 480], DT.bfloat16, tag="ps")
                        for j in range(5):
                            nc.tensor.transpose(
                                psC[:, 96 * j:96 * (j + 1)], sv[:, f0 + j, :],
                                ID[0:96, 0:96]
                            )
                        nc.scalar.copy(dst[:, 96 * f0:96 * (f0 + 5)], psC[:])

                # ---- S5: inverse H FFT (PE-accumulated), packed output ----
                zb = bpool.tile([128, CH], DT.bfloat16, tag="big")  # [h,(c,fpack)]
                zbv = zb[:].rearrange("p (c f) -> p f c", f=128)
                for j in range(13):
                    s = slice(480 * j, 480 * (j + 1))
                    pszr = pspool.tile([128, 480], DT.float32, tag="ps")
                    pszi = pspool.tile([128, 480], DT.float32, tag="ps")
                    nc.tensor.matmul(pszr[:], M["dirt"][:], or_[:, s], start=True, stop=False)
                    nc.tensor.matmul(pszr[:], M["diitn"][:], oi_[:, s], start=False, stop=True)
                    nc.tensor.matmul(pszi[:], M["dirt"][:], oi_[:, s], start=True, stop=False)
                    nc.tensor.matmul(pszi[:], M["diit"][:], or_[:, s], start=False, stop=True)
                    f0 = 5 * j
                    nc.vector.tensor_copy(zbv[:, f0:f0 + 5, :], pszr[:])
                    # imag part -> fpack rows 64+f, dropping f=0 and f=64
                    if j == 0:
                        nc.vector.tensor_copy(zbv[:, 65:69, :], pszi[:, 96:480])
                    elif j == 12:
                        nc.vector.tensor_copy(zbv[:, 124:128, :], pszi[:, 0:384])
                    else:
                        nc.vector.tensor_copy(zbv[:, 64 + f0:69 + f0, :], pszi[:])

                # ---- TD: [h,(c,fpack)] -> ZIpack [fpack,(h,c)] ----
                zp = bpool.tile([128, CH], DT.bfloat16, tag="big")
                zbc = zb[:].rearrange("p (c f) -> p c f", f=128)
                zpv = zp[:].rearrange("p (h c) -> p c h", c=C)
                for c0 in range(0, C, 4):
                    psD = pspool.tile([128, 512], DT.bfloat16, tag="ps")
                    for j in range(4):
                        nc.tensor.transpose(
                            psD[:, 128 * j:128 * (j + 1)], zbc[:, c0 + j, :], ID[:]
                        )
                    nc.scalar.copy(zpv[:, c0:c0 + 4, :], psD[:])

                # ---- S6: inverse W rfft + residual ----
                xrv = xres[b].rearrange("h w c -> w h c")
                orv = out_ext[b].rearrange("h w c -> w h c")
                for j in range(26):
                    lo = 480 * j
                    hi = min(lo + 480, CH)
                    n = hi - lo
                    h0, h1 = lo // C, hi // C
                    ps6 = pspool.tile([128, n], DT.float32, tag="ps")
                    nc.tensor.matmul(ps6[:], M["apack"][:], zp[:, lo:hi],
                                     start=True, stop=True)
                    xr = iopool.tile([128, 480], DT.float32, tag="xr")
                    nc.sync.dma_start(xr[:, :n].rearrange("w (h c) -> w h c", c=C),
                                      xrv[:, h0:h1, :])
                    oc = iopool.tile([128, 480], DT.float32, tag="oc")
                    nc.vector.tensor_add(oc[:, :n], ps6[:], xr[:, :n])
                    nc.sync.dma_start(orv[:, h0:h1, :],
                                      oc[:, :n].rearrange("w (h c) -> w h c", c=C))

    nc.compile()
    return nc


def _get_compiled():
    if "nc" not in _CACHE:
        _CACHE["mats"] = _host_matrices()
        _CACHE["nc"] = _build_nc()
    return _CACHE["nc"], _CACHE["mats"]


def make_in_maps(x, w1, b1, w2, b2):
    mats = _get_compiled()[1]
    in_maps = []
    for i in range(N_CORES):
        sl = slice(C * i, C * (i + 1))
        xs = x[:, :, :, sl]
        m = dict(mats)
        m["x16"] = np.ascontiguousarray(
            xs.transpose(0, 2, 3, 1).astype(BF16))  # [B, w, c, h]
        m["xres"] = np.ascontiguousarray(xs)
        m["w1r"] = np.ascontiguousarray(w1[0, i].astype(BF16))
        m["w1i"] = np.ascontiguousarray(w1[1, i].astype(BF16))
        m["w1in"] = np.ascontiguousarray((-w1[1, i]).astype(BF16))
        m["w2r"] = np.ascontiguousarray(w2[0, i].astype(BF16))
        m["w2i"] = np.ascontiguousarray(w2[1, i].astype(BF16))
        m["w2in"] = np.ascontiguousarray((-w2[1, i]).astype(BF16))
        m["b1r"] = np.ascontiguousarray(b1[0, i].astype(np.float32)[:, None])
        m["b1i"] = np.ascontiguousarray(b1[1, i].astype(np.float32)[:, None])
        m["b2r"] = np.ascontiguousarray(b2[0, i].astype(np.float32)[:, None])
        m["b2i"] = np.ascontiguousarray(b2[1, i].astype(np.float32)[:, None])
        in_maps.append(m)
    return in_maps


def run(x, w1, b1, w2, b2, trace=False):
    nc, _ = _get_compiled()
    in_maps = make_in_maps(x, w1, b1, w2, b2)
    res = run_bass_kernel_spmd(nc, in_maps, core_ids=list(range(N_CORES)),
                               trace=trace)
    out = np.empty((B, H, W, C * N_CORES), dtype=np.float32)
    for i in range(N_CORES):
        out[:, :, :, C * i:C * (i + 1)] = res.results[i]["out"]
    return out, res


def kernel(x, w1, b1, w2, b2):
    x = np.asarray(x, dtype=np.float32)
    out, _ = run(x, np.asarray(w1), np.asarray(b1), np.asarray(w2),
                 np.asarray(b2))
    return out


# revision 7
# speedup vs baseline: 11.3100x; 11.3100x over previous
"""AFNO2D layer on 8 Trainium2 NeuronCores.

Sharding: channel-block parallel. C=768 = 8 blocks x 96 channels; the complex
MLP is block-diagonal over exactly these blocks, and the 2D FFT is independent
per channel — so core i handles channel block i end-to-end with zero
collectives.

Per-core pipeline (per batch b, all DFTs as dense matmuls on the 128x128 PE):
  S1  W-axis rfft, Hermitian-packed:  lhsT=Fpack[w,128] -> Ypack[fpack,(c,h)]
  TA  PE-transpose corner turn    -> Yt[h,(f,c)]
  S2  H-axis complex FFT (2 real matmuls + DVE combines) -> Zr,Zi [g,(f,c)]
  TB  PE-transpose                -> Zrt,Zit [c,(f,g)]
  L1  complex 96x96 matmul + bias + ReLU (PSUM accumulate pairs)
  L2  complex 96x96 matmul + bias; softshrink on DVE
  TC  PE-transpose                -> Or,Oi [g,(f,c)]
  S5  inverse H FFT + combines, packed -> ZIboth [h,(c,fpack)]
  TD  PE-transpose                -> ZIpack [fpack,(h,c)]
  S6  inverse W rfft (packed lhsT=Apack) + residual add -> out
Matmul dtype bf16 (PSUM accumulates f32); residual path f32. The output is
x + delta with ||delta||/||x|| ~ 0.05, so bf16 error in delta is attenuated
~20x in the final rel-err.
"""

import sys

import numpy as np

try:
    import concourse  # noqa: F401
except ImportError:
    sys.path.insert(0, "/opt/trn_rl_repo")

import ml_dtypes

import concourse.bass as bass
import concourse.bacc as bacc
import concourse.mybir as mybir
import concourse.tile as tile
from concourse.bass_utils import run_bass_kernel_spmd

BF16 = ml_dtypes.bfloat16
DT = mybir.dt

B = 4
H = 128
W = 128
C = 96  # per-core channels (one MLP block)
F = 65  # rfft freqs along W
LAM = 0.01
N_CORES = 8

_CACHE = {}


def _host_matrices():
    """DFT matrices, all as matmul lhsT ([K, M]) layouts, bf16."""
    I = np.eye(W)
    R = np.fft.rfft(I, axis=1, norm="ortho")  # [w, f]: Y = x @ R
    fpack = np.concatenate([R.real, R.imag[:, 1:64]], axis=1)  # [w, 128]
    Dm = np.fft.fft(np.eye(H), axis=1, norm="ortho")  # [h, g]: Z = Y @ Dm
    DmI = np.fft.ifft(np.eye(H), axis=1, norm="ortho")  # [g, h]
    Ar = np.zeros((F, W))
    Ai = np.zeros((F, W))
    for f in range(F):
        e = np.zeros(F, dtype=complex)
        e[f] = 1.0
        Ar[f] = np.fft.irfft(e, n=W, norm="ortho")
        e = np.zeros(F, dtype=complex)
        e[f] = 1j
        Ai[f] = np.fft.irfft(e, n=W, norm="ortho")
    apack = np.concatenate([Ar, Ai[1:64]], axis=0)  # [fpack, w]
    c = lambda a: np.ascontiguousarray(a.astype(BF16))
    return {
        "fpack": c(fpack),
        "drt": c(Dm.real),
        "dit": c(Dm.imag),
        "ditn": c(-Dm.imag),
        "dirt": c(DmI.real),
        "diit": c(DmI.imag),
        "diitn": c(-DmI.imag),
        "apack": c(apack),
        "ident": c(np.eye(128)),
    }


def _build_nc(reps=1):
    nc = bacc.Bacc(
        "TRN2", target_bir_lowering=False, debug=False, num_devices=N_CORES
    )

    def din(name, shape, dt):
        return nc.dram_tensor(name, shape, dt, kind="ExternalInput")

    x16 = din("x16", [B, W, C, H], DT.bfloat16)
    xres = din("xres", [B, H, W, C], DT.float32)
    mats = {
        k: din(k, [128, 128], DT.bfloat16)
        for k in ["fpack", "drt", "dit", "ditn", "dirt", "diit", "diitn", "apack", "ident"]
    }
    wts = {k: din(k, [C, C], DT.bfloat16)
           for k in ["w1r", "w1i", "w1in", "w2r", "w2i", "w2in"]}
    bs = {k: din(k, [C, 1], DT.float32) for k in ["b1r", "b1i", "b2r", "b2i"]}
    out_ext = nc.dram_tensor("out", [B, H, W, C], DT.float32, kind="ExternalOutput")

    CH = C * H          # 12288
    FC = F * C          # 6240
    FG = F * 128        # 8320

    with tile.TileContext(nc) as tc:
        with (
            tc.tile_pool(name="const", bufs=1) as cpool,
            tc.tile_pool(name="big", bufs=3) as bpool,
            tc.tile_pool(name="pa", bufs=3) as papool,
            tc.tile_pool(name="pb", bufs=4) as pbpool,
            tc.tile_pool(name="sm", bufs=2) as spool,
            tc.tile_pool(name="io", bufs=2) as iopool,
            tc.tile_pool(name="ps", bufs=8, space="PSUM") as pspool,
        ):
            M = {}
            for k in mats:
                M[k] = cpool.tile([128, 128], DT.bfloat16, tag=f"m_{k}", name=f"m_{k}")
                nc.sync.dma_start(M[k][:], mats[k][:])
            Wt = {}
            for k in wts:
                Wt[k] = cpool.tile([C, C], DT.bfloat16, tag=f"w_{k}", name=f"wt_{k}")
                nc.sync.dma_start(Wt[k][:], wts[k][:])
            Bt = {}
            for k in bs:
                Bt[k] = cpool.tile([C, 1], DT.float32, tag=f"b_{k}", name=f"bt_{k}")
                nc.sync.dma_start(Bt[k][:], bs[k][:])
            ID = M["ident"]

            for b in [bb for _ in range(reps) for bb in range(B)]:
                # ---- load x (pre-packed bf16 [w, (c, h)]) ----
                xb = bpool.tile([128, CH], DT.bfloat16, tag="big")
                nc.sync.dma_start(xb[:], x16[b].rearrange("w c h -> w (c h)"))

                # ---- S1: W-rfft packed ----
                yp = bpool.tile([128, CH], DT.bfloat16, tag="big")
                for k in range(CH // 512):
                    s = slice(512 * k, 512 * (k + 1))
                    ps = pspool.tile([128, 512], DT.float32, tag="ps")
                    nc.tensor.matmul(ps[:], M["fpack"][:], xb[:, s],
                                     start=True, stop=True)
                    nc.scalar.copy(yp[:, s], ps[:])

                # ---- TA: [fpack,(c,h)] -> Yt [h,(f,c)] ----
                yt = bpool.tile([128, CH], DT.bfloat16, tag="big")
                ypv = yp[:].rearrange("p (c h) -> p c h", h=H)
                ytv = yt[:].rearrange("p (f c) -> p c f", c=C)
                for c0 in range(0, C, 4):
                    psT = pspool.tile([128, 512], DT.bfloat16, tag="ps")
                    for j in range(4):
                        nc.tensor.transpose(
                            psT[:, 128 * j:128 * (j + 1)], ypv[:, c0 + j, :], ID[:]
                        )
                    nc.scalar.copy(ytv[:, c0:c0 + 4, :], psT[:])

                # ---- S2: H-axis complex FFT (PE-accumulated combines) ----
                zr = papool.tile([128, FC], DT.bfloat16, tag="pa")
                zi = papool.tile([128, FC], DT.bfloat16, tag="pa")
                # edges f=0 (cols 0:96) and f=64 (cols 6144:6240): Yi=0 there
                for cols in [slice(0, 96), slice(6144, 6240)]:
                    pe1 = pspool.tile([128, 96], DT.float32, tag="ps")
                    pe2 = pspool.tile([128, 96], DT.float32, tag="ps")
                    nc.tensor.matmul(pe1[:], M["drt"][:], yt[:, cols],
                                     start=True, stop=True)
                    nc.tensor.matmul(pe2[:], M["dit"][:], yt[:, cols],
                                     start=True, stop=True)
                    nc.scalar.copy(zr[:, cols], pe1[:])
                    nc.scalar.copy(zi[:, cols], pe2[:])
                for j in range(12):
                    sa = slice(96 + 504 * j, 96 + 504 * (j + 1))      # fr f=1..63
                    sb = slice(6240 + 504 * j, 6240 + 504 * (j + 1))  # fi f=1..63
                    pszr = pspool.tile([128, 504], DT.float32, tag="ps")
                    pszi = pspool.tile([128, 504], DT.float32, tag="ps")
                    nc.tensor.matmul(pszr[:], M["drt"][:], yt[:, sa], start=True, stop=False)
                    nc.tensor.matmul(pszr[:], M["ditn"][:], yt[:, sb], start=False, stop=True)
                    nc.tensor.matmul(pszi[:], M["drt"][:], yt[:, sb], start=True, stop=False)
                    nc.tensor.matmul(pszi[:], M["dit"][:], yt[:, sa], start=False, stop=True)
                    nc.vector.tensor_copy(zr[:, sa], pszr[:])
                    nc.vector.tensor_copy(zi[:, sa], pszi[:])

                # ---- TB: [g,(f,c)] -> [c,(f,g)] ----
                zrt = pbpool.tile([C, FG], DT.bfloat16, tag="pb")
                zit = pbpool.tile([C, FG], DT.bfloat16, tag="pb")
                for (src, dst) in [(zr, zrt), (zi, zit)]:
                    sv = src[:].rearrange("p (f c) -> p f c", c=C)
                    for f0 in range(0, 64, 4):
                        psB = pspool.tile([C, 512], DT.bfloat16, tag="ps")
                        for j in range(4):
                            nc.tensor.transpose(
                                psB[:, 128 * j:128 * (j + 1)], sv[:, f0 + j, :], ID[:]
                            )
                        nc.scalar.copy(dst[:, 128 * f0:128 * (f0 + 4)], psB[:])
                    psB = pspool.tile([C, 128], DT.bfloat16, tag="ps")
                    nc.tensor.transpose(psB[:], sv[:, 64, :], ID[:])
                    nc.scalar.copy(dst[:, 128 * 64:128 * 65], psB[:])

                # ---- L1 + L2 MLP fused (chunks over (f,g)) ----
                o2r = pbpool.tile([C, FG], DT.bfloat16, tag="pb")
                o2i = pbpool.tile([C, FG], DT.bfloat16, tag="pb")
                chunks = [slice(512 * k, min(512 * (k + 1), FG))
                          for k in range((FG + 511) // 512)]
                for s in chunks:
                    n = s.stop - s.start
                    psr = pspool.tile([C, n], DT.float32, tag="ps")
                    psi = pspool.tile([C, n], DT.float32, tag="ps")
                    nc.tensor.matmul(psr[:], Wt["w1r"][:], zrt[:, s], start=True, stop=False)
                    nc.tensor.matmul(psr[:], Wt["w1in"][:], zit[:, s], start=False, stop=True)
                    nc.tensor.matmul(psi[:], Wt["w1r"][:], zit[:, s], start=True, stop=False)
                    nc.tensor.matmul(psi[:], Wt["w1i"][:], zrt[:, s], start=False, stop=True)
                    o1rc = spool.tile([C, 512], DT.bfloat16, tag="o1r", name="o1rc")
                    o1ic = spool.tile([C, 512], DT.bfloat16, tag="o1i", name="o1ic")
                    nc.scalar.activation(o1rc[:, :n], psr[:],
                                         mybir.ActivationFunctionType.Relu,
                                         bias=Bt["b1r"][:, 0:1])
                    nc.scalar.activation(o1ic[:, :n], psi[:],
                                         mybir.ActivationFunctionType.Relu,
                                         bias=Bt["b1i"][:, 0:1])
                    psr2 = pspool.tile([C, n], DT.float32, tag="ps")
                    psi2 = pspool.tile([C, n], DT.float32, tag="ps")
                    nc.tensor.matmul(psr2[:], Wt["w2r"][:], o1rc[:, :n], start=True, stop=False)
                    nc.tensor.matmul(psr2[:], Wt["w2in"][:], o1ic[:, :n], start=False, stop=True)
                    nc.tensor.matmul(psi2[:], Wt["w2r"][:], o1ic[:, :n], start=True, stop=False)
                    nc.tensor.matmul(psi2[:], Wt["w2i"][:], o1rc[:, :n], start=False, stop=True)
                    for ps, bias, dst in [(psr2, "b2r", o2r), (psi2, "b2i", o2i)]:
                        t = spool.tile([C, 512], DT.bfloat16, tag="ss1", name="sst")
                        tcl = spool.tile([C, 512], DT.bfloat16, tag="ss2", name="sscl")
                        nc.scalar.activation(t[:, :n], ps[:],
                                             mybir.ActivationFunctionType.Identity,
                                             bias=Bt[bias][:, 0:1])
                        nc.vector.tensor_scalar(
                            tcl[:, :n], t[:, :n], -LAM, LAM,
                            mybir.AluOpType.max, mybir.AluOpType.min)
                        nc.vector.tensor_sub(dst[:, s], t[:, :n], tcl[:, :n])

                # ---- TC: [c,(f,g)] -> [g,(f,c)] ----
                or_ = papool.tile([128, FC], DT.bfloat16, tag="pa")
                oi_ = papool.tile([128, FC], DT.bfloat16, tag="pa")
                for (src, dst) in [(o2r, or_), (o2i, oi_)]:
                    sv = src[:].rearrange("p (f g) -> p f g", g=128)
                    for f0 in range(0, F, 5):
                        psC = pspool.tile([128, 480], DT.bfloat16, tag="ps")
                        for j in range(5):
                            nc.tensor.transpose(
                                psC[:, 96 * j:96 * (j + 1)], sv[:, f0 + j, :],
                                ID[0:96, 0:96]
                            )
                        nc.scalar.copy(dst[:, 96 * f0:96 * (f0 + 5)], psC[:])

                # ---- S5: inverse H FFT (PE-accumulated), packed output ----
                zb = bpool.tile([128, CH], DT.bfloat16, tag="big")  # [h,(c,fpack)]
                zbv = zb[:].rearrange("p (c f) -> p f c", f=128)
                for j in range(13):
                    s = slice(480 * j, 480 * (j + 1))
                    pszr = pspool.tile([128, 480], DT.float32, tag="ps")
                    pszi = pspool.tile([128, 480], DT.float32, tag="ps")
                    nc.tensor.matmul(pszr[:], M["dirt"][:], or_[:, s], start=True, stop=False)
                    nc.tensor.matmul(pszr[:], M["diitn"][:], oi_[:, s], start=False, stop=True)
                    nc.tensor.matmul(pszi[:], M["dirt"][:], oi_[:, s], start=True, stop=False)
                    nc.tensor.matmul(pszi[:], M["diit"][:], or_[:, s], start=False, stop=True)
                    f0 = 5 * j
                    nc.vector.tensor_copy(zbv[:, f0:f0 + 5, :], pszr[:])
                    # imag part -> fpack rows 64+f, dropping f=0 and f=64
                    if j == 0:
                        nc.vector.tensor_copy(zbv[:, 65:69, :], pszi[:, 96:480])
                    elif j == 12:
                        nc.vector.tensor_copy(zbv[:, 124:128, :], pszi[:, 0:384])
                    else:
                        nc.vector.tensor_copy(zbv[:, 64 + f0:69 + f0, :], pszi[:])

                # ---- TD: [h,(c,fpack)] -> ZIpack [fpack,(h,c)] ----
                zp = bpool.tile([128, CH], DT.bfloat16, tag="big")
                zbc = zb[:].rearrange("p (c f) -> p c f", f=128)
                zpv = zp[:].rearrange("p (h c) -> p c h", c=C)
                for c0 in range(0, C, 4):
                    psD = pspool.tile([128, 512], DT.bfloat16, tag="ps")
                    for j in range(4):
                        nc.tensor.transpose(
                            psD[:, 128 * j:128 * (j + 1)], zbc[:, c0 + j, :], ID[:]
                        )
                    nc.scalar.copy(zpv[:, c0:c0 + 4, :], psD[:])

                # ---- S6: inverse W rfft + residual ----
                xrv = xres[b].rearrange("h w c -> w h c")
                orv = out_ext[b].rearrange("h w c -> w h c")
                for j in range(26):
                    lo = 480 * j
                    hi = min(lo + 480, CH)
                    n = hi - lo
                    h0, h1 = lo // C, hi // C
                    ps6 = pspool.tile([128, n], DT.float32, tag="ps")
                    nc.tensor.matmul(ps6[:], M["apack"][:], zp[:, lo:hi],
                                     start=True, stop=True)
                    xr = iopool.tile([128, 480], DT.float32, tag="xr")
                    nc.sync.dma_start(xr[:, :n].rearrange("w (h c) -> w h c", c=C),
                                      xrv[:, h0:h1, :])
                    oc = iopool.tile([128, 480], DT.float32, tag="oc")
                    nc.vector.tensor_add(oc[:, :n], ps6[:], xr[:, :n])
                    nc.sync.dma_start(orv[:, h0:h1, :],
                                      oc[:, :n].rearrange("w (h c) -> w h c", c=C))

    nc.compile()
    return nc


def _get_compiled(reps=1):
    key = f"nc{reps}"
    if key not in _CACHE:
        if "mats" not in _CACHE:
            _CACHE["mats"] = _host_matrices()
        _CACHE[key] = _build_nc(reps)
    return _CACHE[key], _CACHE["mats"]


def make_in_maps(x, w1, b1, w2, b2):
    mats = _get_compiled()[1]
    in_maps = []
    for i in range(N_CORES):
        sl = slice(C * i, C * (i + 1))
        xs = x[:, :, :, sl]
        m = dict(mats)
        m["x16"] = np.ascontiguousarray(
            xs.transpose(0, 2, 3, 1).astype(BF16))  # [B, w, c, h]
        m["xres"] = np.ascontiguousarray(xs)
        m["w1r"] = np.ascontiguousarray(w1[0, i].astype(BF16))
        m["w1i"] = np.ascontiguousarray(w1[1, i].astype(BF16))
        m["w1in"] = np.ascontiguousarray((-w1[1, i]).astype(BF16))
        m["w2r"] = np.ascontiguousarray(w2[0, i].astype(BF16))
        m["w2i"] = np.ascontiguousarray(w2[1, i].astype(BF16))
        m["w2in"] = np.ascontiguousarray((-w2[1, i]).astype(BF16))
        m["b1r"] = np.ascontiguousarray(b1[0, i].astype(np.float32)[:, None])
        m["b1i"] = np.ascontiguousarray(b1[1, i].astype(np.float32)[:, None])
        m["b2r"] = np.ascontiguousarray(b2[0, i].astype(np.float32)[:, None])
        m["b2i"] = np.ascontiguousarray(b2[1, i].astype(np.float32)[:, None])
        in_maps.append(m)
    return in_maps


def run(x, w1, b1, w2, b2, trace=False):
    nc, _ = _get_compiled()
    in_maps = make_in_maps(x, w1, b1, w2, b2)
    res = run_bass_kernel_spmd(nc, in_maps, core_ids=list(range(N_CORES)),
                               trace=trace)
    out = np.empty((B, H, W, C * N_CORES), dtype=np.float32)
    for i in range(N_CORES):
        out[:, :, :, C * i:C * (i + 1)] = res.results[i]["out"]
    return out, res


def kernel(x, w1, b1, w2, b2):
    x = np.asarray(x, dtype=np.float32)
    out, _ = run(x, np.asarray(w1), np.asarray(b1), np.asarray(w2),
                 np.asarray(b2))
    return out


# revision 20
# speedup vs baseline: 61.8110x; 5.4652x over previous
"""AFNO2D layer on 8 Trainium2 NeuronCores.

Sharding: channel-block parallel. C=768 = 8 blocks x 96 channels; the complex
MLP is block-diagonal over exactly these blocks, and the 2D FFT is independent
per channel — so core i handles channel block i end-to-end with zero
collectives.

Per-core pipeline (per batch b, all DFTs as dense matmuls on the 128x128 PE):
  S1  W-axis rfft, Hermitian-packed:  lhsT=Fpack[w,128] -> Ypack[fpack,(c,h)]
  TA  PE-transpose corner turn    -> Yt[h,(f,c)]
  S2  H-axis complex FFT (2 real matmuls + DVE combines) -> Zr,Zi [g,(f,c)]
  TB  PE-transpose                -> Zrt,Zit [c,(f,g)]
  L1  complex 96x96 matmul + bias + ReLU (PSUM accumulate pairs)
  L2  complex 96x96 matmul + bias; softshrink on DVE
  TC  PE-transpose                -> Or,Oi [g,(f,c)]
  S5  inverse H FFT + combines, packed -> ZIboth [h,(c,fpack)]
  TD  PE-transpose                -> ZIpack [fpack,(h,c)]
  S6  inverse W rfft (packed lhsT=Apack) + residual add -> out
Matmul dtype bf16 (PSUM accumulates f32); residual path f32. The output is
x + delta with ||delta||/||x|| ~ 0.05, so bf16 error in delta is attenuated
~20x in the final rel-err.
"""

import sys

import numpy as np

try:
    import concourse  # noqa: F401
except ImportError:
    sys.path.insert(0, "/opt/trn_rl_repo")

import ml_dtypes

import concourse.bass as bass
import concourse.bacc as bacc
import concourse.mybir as mybir
import concourse.tile as tile
from concourse.bass_utils import run_bass_kernel_spmd

BF16 = ml_dtypes.bfloat16
DT = mybir.dt

B = 4
H = 128
W = 128
C = 96  # per-core channels (one MLP block)
F = 65  # rfft freqs along W
LAM = 0.01
N_CORES = 8

_CACHE = {}


def _host_matrices():
    """DFT matrices, all as matmul lhsT ([K, M]) layouts, bf16."""
    I = np.eye(W)
    R = np.fft.rfft(I, axis=1, norm="ortho")  # [w, f]: Y = x @ R
    fpack = np.concatenate([R.real, R.imag[:, 1:64]], axis=1)  # [w, 128]
    Dm = np.fft.fft(np.eye(H), axis=1, norm="ortho")  # [h, g]: Z = Y @ Dm
    DmI = np.fft.ifft(np.eye(H), axis=1, norm="ortho")  # [g, h]
    Ar = np.zeros((F, W))
    Ai = np.zeros((F, W))
    for f in range(F):
        e = np.zeros(F, dtype=complex)
        e[f] = 1.0
        Ar[f] = np.fft.irfft(e, n=W, norm="ortho")
        e = np.zeros(F, dtype=complex)
        e[f] = 1j
        Ai[f] = np.fft.irfft(e, n=W, norm="ortho")
    apack = np.concatenate([Ar, Ai[1:64]], axis=0)  # [fpack, w]
    c = lambda a: np.ascontiguousarray(a.astype(BF16))
    return {
        "fpack": c(fpack),
        "drt": c(Dm.real),
        "dit": c(Dm.imag),
        "ditn": c(-Dm.imag),
        "dirt": c(DmI.real),
        "diit": c(DmI.imag),
        "diitn": c(-DmI.imag),
        "apack": c(apack),
        "ident": c(np.eye(128)),
    }


def _build_nc(reps=1, mode="full"):
    if mode.startswith("v2"):
        return _build_nc_v2(reps, mode)
    if mode.startswith("v3"):
        return _build_nc_v3(reps, mode)
    if mode == "tiny":
        return _build_nc_tiny(reps)
    nc = bacc.Bacc(
        "TRN2", target_bir_lowering=False, debug=False, num_devices=N_CORES
    )

    def din(name, shape, dt):
        return nc.dram_tensor(name, shape, dt, kind="ExternalInput")

    x16 = din("x16", [B, W, C, H], DT.bfloat16)
    xres = din("xres", [B, H, W, C], DT.float32)
    mats = {
        k: din(k, [128, 128], DT.bfloat16)
        for k in ["fpack", "drt", "dit", "ditn", "dirt", "diit", "diitn", "apack", "ident"]
    }
    wts = {k: din(k, [C, C], DT.bfloat16)
           for k in ["w1r", "w1i", "w1in", "w2r", "w2i", "w2in"]}
    bs = {k: din(k, [C, 1], DT.float32) for k in ["b1r", "b1i", "b2r", "b2i"]}
    out_ext = nc.dram_tensor("out", [B, H, W, C], DT.float32, kind="ExternalOutput")

    CH = C * H          # 12288
    FC = F * C          # 6240
    FG = F * 128        # 8320

    with tile.TileContext(nc) as tc:
        with (
            tc.tile_pool(name="const", bufs=1) as cpool,
            tc.tile_pool(name="big", bufs=3) as bpool,
            tc.tile_pool(name="pa", bufs=3) as papool,
            tc.tile_pool(name="pb", bufs=4) as pbpool,
            tc.tile_pool(name="sm", bufs=2) as spool,
            tc.tile_pool(name="io", bufs=2) as iopool,
            tc.tile_pool(name="ps", bufs=8, space="PSUM") as pspool,
        ):
            M = {}
            for k in mats:
                M[k] = cpool.tile([128, 128], DT.bfloat16, tag=f"m_{k}", name=f"m_{k}")
                nc.sync.dma_start(M[k][:], mats[k][:])
            Wt = {}
            for k in wts:
                Wt[k] = cpool.tile([C, C], DT.bfloat16, tag=f"w_{k}", name=f"wt_{k}")
                nc.sync.dma_start(Wt[k][:], wts[k][:])
            Bt = {}
            for k in bs:
                Bt[k] = cpool.tile([C, 1], DT.float32, tag=f"b_{k}", name=f"bt_{k}")
                nc.sync.dma_start(Bt[k][:], bs[k][:])
            ID = M["ident"]

            for b in [bb for _ in range(reps) for bb in range(B)]:
                # ---- load x (pre-packed bf16 [w, (c, h)]) ----
                xb = bpool.tile([128, CH], DT.bfloat16, tag="big")
                nc.sync.dma_start(xb[:], x16[b].rearrange("w c h -> w (c h)"))

                # ---- S1: W-rfft packed ----
                yp = bpool.tile([128, CH], DT.bfloat16, tag="big")
                for k in range(CH // 512):
                    s = slice(512 * k, 512 * (k + 1))
                    ps = pspool.tile([128, 512], DT.float32, tag="ps")
                    nc.tensor.matmul(ps[:], M["fpack"][:], xb[:, s],
                                     start=True, stop=True)
                    nc.scalar.copy(yp[:, s], ps[:])

                # ---- TA: [fpack,(c,h)] -> Yt [h,(f,c)] ----
                yt = bpool.tile([128, CH], DT.bfloat16, tag="big")
                ypv = yp[:].rearrange("p (c h) -> p c h", h=H)
                ytv = yt[:].rearrange("p (f c) -> p c f", c=C)
                for c0 in range(0, C, 4):
                    psT = pspool.tile([128, 512], DT.bfloat16, tag="ps")
                    for j in range(4):
                        nc.tensor.transpose(
                            psT[:, 128 * j:128 * (j + 1)], ypv[:, c0 + j, :], ID[:]
                        )
                    nc.scalar.copy(ytv[:, c0:c0 + 4, :], psT[:])

                # ---- S2: H-axis complex FFT (PE-accumulated combines) ----
                zr = papool.tile([128, FC], DT.bfloat16, tag="pa")
                zi = papool.tile([128, FC], DT.bfloat16, tag="pa")
                # edges f=0 (cols 0:96) and f=64 (cols 6144:6240): Yi=0 there
                for cols in [slice(0, 96), slice(6144, 6240)]:
                    pe1 = pspool.tile([128, 96], DT.float32, tag="ps")
                    pe2 = pspool.tile([128, 96], DT.float32, tag="ps")
                    nc.tensor.matmul(pe1[:], M["drt"][:], yt[:, cols],
                                     start=True, stop=True)
                    nc.tensor.matmul(pe2[:], M["dit"][:], yt[:, cols],
                                     start=True, stop=True)
                    nc.scalar.copy(zr[:, cols], pe1[:])
                    nc.scalar.copy(zi[:, cols], pe2[:])
                for j in range(12):
                    sa = slice(96 + 504 * j, 96 + 504 * (j + 1))      # fr f=1..63
                    sb = slice(6240 + 504 * j, 6240 + 504 * (j + 1))  # fi f=1..63
                    pszr = pspool.tile([128, 504], DT.float32, tag="ps")
                    pszi = pspool.tile([128, 504], DT.float32, tag="ps")
                    nc.tensor.matmul(pszr[:], M["drt"][:], yt[:, sa], start=True, stop=False)
                    nc.tensor.matmul(pszr[:], M["ditn"][:], yt[:, sb], start=False, stop=True)
                    nc.tensor.matmul(pszi[:], M["drt"][:], yt[:, sb], start=True, stop=False)
                    nc.tensor.matmul(pszi[:], M["dit"][:], yt[:, sa], start=False, stop=True)
                    nc.vector.tensor_copy(zr[:, sa], pszr[:])
                    nc.vector.tensor_copy(zi[:, sa], pszi[:])

                # ---- TB: [g,(f,c)] -> [c,(f,g)] ----
                zrt = pbpool.tile([C, FG], DT.bfloat16, tag="pb")
                zit = pbpool.tile([C, FG], DT.bfloat16, tag="pb")
                for (src, dst) in [(zr, zrt), (zi, zit)]:
                    sv = src[:].rearrange("p (f c) -> p f c", c=C)
                    for f0 in range(0, 64, 4):
                        psB = pspool.tile([C, 512], DT.bfloat16, tag="ps")
                        for j in range(4):
                            nc.tensor.transpose(
                                psB[:, 128 * j:128 * (j + 1)], sv[:, f0 + j, :], ID[:]
                            )
                        nc.scalar.copy(dst[:, 128 * f0:128 * (f0 + 4)], psB[:])
                    psB = pspool.tile([C, 128], DT.bfloat16, tag="ps")
                    nc.tensor.transpose(psB[:], sv[:, 64, :], ID[:])
                    nc.scalar.copy(dst[:, 128 * 64:128 * 65], psB[:])

                # ---- L1 + L2 MLP fused (chunks over (f,g)) ----
                o2r = pbpool.tile([C, FG], DT.bfloat16, tag="pb")
                o2i = pbpool.tile([C, FG], DT.bfloat16, tag="pb")
                chunks = [slice(512 * k, min(512 * (k + 1), FG))
                          for k in range((FG + 511) // 512)]
                for s in chunks:
                    n = s.stop - s.start
                    psr = pspool.tile([C, n], DT.float32, tag="ps")
                    psi = pspool.tile([C, n], DT.float32, tag="ps")
                    nc.tensor.matmul(psr[:], Wt["w1r"][:], zrt[:, s], start=True, stop=False)
                    nc.tensor.matmul(psr[:], Wt["w1in"][:], zit[:, s], start=False, stop=True)
                    nc.tensor.matmul(psi[:], Wt["w1r"][:], zit[:, s], start=True, stop=False)
                    nc.tensor.matmul(psi[:], Wt["w1i"][:], zrt[:, s], start=False, stop=True)
                    o1rc = spool.tile([C, 512], DT.bfloat16, tag="o1r", name="o1rc")
                    o1ic = spool.tile([C, 512], DT.bfloat16, tag="o1i", name="o1ic")
                    nc.scalar.activation(o1rc[:, :n], psr[:],
                                         mybir.ActivationFunctionType.Relu,
                                         bias=Bt["b1r"][:, 0:1])
                    nc.scalar.activation(o1ic[:, :n], psi[:],
                                         mybir.ActivationFunctionType.Relu,
                                         bias=Bt["b1i"][:, 0:1])
                    psr2 = pspool.tile([C, n], DT.float32, tag="ps")
                    psi2 = pspool.tile([C, n], DT.float32, tag="ps")
                    nc.tensor.matmul(psr2[:], Wt["w2r"][:], o1rc[:, :n], start=True, stop=False)
                    nc.tensor.matmul(psr2[:], Wt["w2in"][:], o1ic[:, :n], start=False, stop=True)
                    nc.tensor.matmul(psi2[:], Wt["w2r"][:], o1ic[:, :n], start=True, stop=False)
                    nc.tensor.matmul(psi2[:], Wt["w2i"][:], o1rc[:, :n], start=False, stop=True)
                    for ps, bias, dst in [(psr2, "b2r", o2r), (psi2, "b2i", o2i)]:
                        t = spool.tile([C, 512], DT.bfloat16, tag="ss1", name="sst")
                        tcl = spool.tile([C, 512], DT.bfloat16, tag="ss2", name="sscl")
                        nc.scalar.activation(t[:, :n], ps[:],
                                             mybir.ActivationFunctionType.Identity,
                                             bias=Bt[bias][:, 0:1])
                        nc.vector.tensor_scalar(
                            tcl[:, :n], t[:, :n], -LAM, LAM,
                            mybir.AluOpType.max, mybir.AluOpType.min)
                        nc.vector.tensor_sub(dst[:, s], t[:, :n], tcl[:, :n])

                # ---- TC: [c,(f,g)] -> [g,(f,c)] ----
                or_ = papool.tile([128, FC], DT.bfloat16, tag="pa")
                oi_ = papool.tile([128, FC], DT.bfloat16, tag="pa")
                for (src, dst) in [(o2r, or_), (o2i, oi_)]:
                    sv = src[:].rearrange("p (f g) -> p f g", g=128)
                    for f0 in range(0, F, 5):
                        psC = pspool.tile([128, 480], DT.bfloat16, tag="ps")
                        for j in range(5):
                            nc.tensor.transpose(
                                psC[:, 96 * j:96 * (j + 1)], sv[:, f0 + j, :],
                                ID[0:96, 0:96]
                            )
                        nc.scalar.copy(dst[:, 96 * f0:96 * (f0 + 5)], psC[:])

                # ---- S5: inverse H FFT (PE-accumulated), packed output ----
                zb = bpool.tile([128, CH], DT.bfloat16, tag="big")  # [h,(c,fpack)]
                zbv = zb[:].rearrange("p (c f) -> p f c", f=128)
                for j in range(13):
                    s = slice(480 * j, 480 * (j + 1))
                    pszr = pspool.tile([128, 480], DT.float32, tag="ps")
                    pszi = pspool.tile([128, 480], DT.float32, tag="ps")
                    nc.tensor.matmul(pszr[:], M["dirt"][:], or_[:, s], start=True, stop=False)
                    nc.tensor.matmul(pszr[:], M["diitn"][:], oi_[:, s], start=False, stop=True)
                    nc.tensor.matmul(pszi[:], M["dirt"][:], oi_[:, s], start=True, stop=False)
                    nc.tensor.matmul(pszi[:], M["diit"][:], or_[:, s], start=False, stop=True)
                    f0 = 5 * j
                    nc.vector.tensor_copy(zbv[:, f0:f0 + 5, :], pszr[:])
                    # imag part -> fpack rows 64+f, dropping f=0 and f=64
                    if j == 0:
                        nc.vector.tensor_copy(zbv[:, 65:69, :], pszi[:, 96:480])
                    elif j == 12:
                        nc.vector.tensor_copy(zbv[:, 124:128, :], pszi[:, 0:384])
                    else:
                        nc.vector.tensor_copy(zbv[:, 64 + f0:69 + f0, :], pszi[:])

                # ---- TD: [h,(c,fpack)] -> ZIpack [fpack,(h,c)] ----
                zp = bpool.tile([128, CH], DT.bfloat16, tag="big")
                zbc = zb[:].rearrange("p (c f) -> p c f", f=128)
                zpv = zp[:].rearrange("p (h c) -> p c h", c=C)
                for c0 in range(0, C, 4):
                    psD = pspool.tile([128, 512], DT.bfloat16, tag="ps")
                    for j in range(4):
                        nc.tensor.transpose(
                            psD[:, 128 * j:128 * (j + 1)], zbc[:, c0 + j, :], ID[:]
                        )
                    nc.scalar.copy(zpv[:, c0:c0 + 4, :], psD[:])

                # ---- S6: inverse W rfft + residual ----
                xrv = xres[b].rearrange("h w c -> w h c")
                orv = out_ext[b].rearrange("h w c -> w h c")
                xbv = xb[:].rearrange("p (c h) -> p h c", h=H)
                for j in range(26):
                    lo = 480 * j
                    hi = min(lo + 480, CH)
                    n = hi - lo
                    h0, h1 = lo // C, hi // C
                    ps6 = pspool.tile([128, n], DT.float32, tag="ps")
                    nc.tensor.matmul(ps6[:], M["apack"][:], zp[:, lo:hi],
                                     start=True, stop=True)
                    oc = iopool.tile([128, 480], DT.float32, tag="oc")
                    if mode == "full":
                        xr = iopool.tile([128, 480], DT.float32, tag="xr")
                        nc.sync.dma_start(
                            xr[:, :n].rearrange("w (h c) -> w h c", c=C),
                            xrv[:, h0:h1, :])
                        nc.vector.tensor_add(oc[:, :n], ps6[:], xr[:, :n])
                    else:
                        nc.vector.tensor_add(
                            oc[:, :n].rearrange("w (h c) -> w h c", c=C),
                            ps6[:].rearrange("w (h c) -> w h c", c=C),
                            xbv[:, h0:h1, :])
                    if mode != "nos6dma":
                        nc.sync.dma_start(orv[:, h0:h1, :],
                                          oc[:, :n].rearrange("w (h c) -> w h c", c=C))

    nc.compile()
    return nc


def _build_nc_v3(reps=1, mode="v3"):
    noturns = "noturns" in mode
    """v3 = v1 structure (PE turns) with:
    - softshrink as relu(u-lam) - relu(-u-lam), folded into the ACT bias
      (kills the slow dual-scalar DVE ops)
    - wider bf16 PSUM tiles for turn evictions (fewer, bigger ACT/DVE ops)
    - eviction work split across ACT and DVE
    """
    nc = bacc.Bacc(
        "TRN2", target_bir_lowering=False, debug=False, num_devices=N_CORES
    )

    def din(name, shape, dt):
        return nc.dram_tensor(name, shape, dt, kind="ExternalInput")

    x16 = din("x16", [B, W, C, H], DT.bfloat16)
    xres = din("xres", [B, H, W, C], DT.float32)
    mats = {
        k: din(k, [128, 128], DT.bfloat16)
        for k in ["fpack", "drt", "dit", "ditn", "dirt", "diit", "diitn",
                  "apack", "ident"]
    }
    wts = {k: din(k, [C, C], DT.bfloat16)
           for k in ["w1r", "w1i", "w1in", "w2r", "w2i", "w2in"]}
    bias_names = ["b1r", "b1i", "b2rm", "b2rp", "b2im", "b2ip"]
    bs = {k: din(k, [C, 1], DT.float32) for k in bias_names}
    out_ext = nc.dram_tensor("out", [B, H, W, C], DT.float32, kind="ExternalOutput")

    CH = C * H
    FC = F * C
    FG = F * 128

    with tile.TileContext(nc) as tc:
        with (
            tc.tile_pool(name="const", bufs=1) as cpool,
            tc.tile_pool(name="big", bufs=3) as bpool,
            tc.tile_pool(name="pa", bufs=3) as papool,
            tc.tile_pool(name="pb", bufs=4) as pbpool,
            tc.tile_pool(name="sm", bufs=2) as spool,
            tc.tile_pool(name="io", bufs=2) as iopool,
            tc.tile_pool(name="ps", bufs=6, space="PSUM") as pspool,
            tc.tile_pool(name="pst", bufs=2, space="PSUM") as pstpool,
        ):
            M = {}
            for k in mats:
                M[k] = cpool.tile([128, 128], DT.bfloat16, tag=f"m_{k}", name=f"m_{k}")
                nc.sync.dma_start(M[k][:], mats[k][:])
            Wt = {}
            for k in wts:
                Wt[k] = cpool.tile([C, C], DT.bfloat16, tag=f"w_{k}", name=f"wt_{k}")
                nc.sync.dma_start(Wt[k][:], wts[k][:])
            Bt = {}
            for k in bs:
                Bt[k] = cpool.tile([C, 1], DT.float32, tag=f"b_{k}", name=f"bt_{k}")
                nc.sync.dma_start(Bt[k][:], bs[k][:])
            ID = M["ident"]

            for b in [bb for _ in range(reps) for bb in range(B)]:
                xb = bpool.tile([128, CH], DT.bfloat16, tag="big")
                nc.sync.dma_start(xb[:], x16[b].rearrange("w c h -> w (c h)"))

                # ---- S1: W-rfft packed; evict on DVE ----
                yp = bpool.tile([128, CH], DT.bfloat16, tag="big")
                for k in range(CH // 512):
                    s = slice(512 * k, 512 * (k + 1))
                    ps = pspool.tile([128, 512], DT.float32, tag="ps")
                    nc.tensor.matmul(ps[:], M["fpack"][:], xb[:, s],
                                     start=True, stop=True)
                    nc.vector.tensor_copy(yp[:, s], ps[:])

                # ---- TA: [fpack,(c,h)] -> Yt [h,(f,c)]; 8-c groups ----
                yt = bpool.tile([128, CH], DT.bfloat16, tag="big")
                ypv = yp[:].rearrange("p (c h) -> p c h", h=H)
                ytv = yt[:].rearrange("p (f c) -> p c f", c=C)
                if noturns:
                    for k in range(CH // 1024):
                        s = slice(1024 * k, 1024 * (k + 1))
                        nc.vector.tensor_copy(yt[:, s], yp[:, s])
                else:
                    for c0 in range(0, C, 8):
                        psT = pstpool.tile([128, 1024], DT.bfloat16, tag="pst")
                        for j in range(8):
                            nc.tensor.transpose(
                                psT[:, 128 * j:128 * (j + 1)], ypv[:, c0 + j, :], ID[:]
                            )
                        nc.vector.tensor_copy(ytv[:, c0:c0 + 8, :], psT[:])

                # ---- S2: H-axis complex FFT; evicts on DVE ----
                zr = papool.tile([128, FC], DT.bfloat16, tag="pa")
                zi = papool.tile([128, FC], DT.bfloat16, tag="pa")
                for cols in [slice(0, 96), slice(6144, 6240)]:
                    pe1 = pspool.tile([128, 96], DT.float32, tag="ps")
                    pe2 = pspool.tile([128, 96], DT.float32, tag="ps")
                    nc.tensor.matmul(pe1[:], M["drt"][:], yt[:, cols],
                                     start=True, stop=True)
                    nc.tensor.matmul(pe2[:], M["dit"][:], yt[:, cols],
                                     start=True, stop=True)
                    nc.vector.tensor_copy(zr[:, cols], pe1[:])
                    nc.vector.tensor_copy(zi[:, cols], pe2[:])
                for j in range(12):
                    sa = slice(96 + 504 * j, 96 + 504 * (j + 1))
                    sb = slice(6240 + 504 * j, 6240 + 504 * (j + 1))
                    pszr = pspool.tile([128, 504], DT.float32, tag="ps")
                    pszi = pspool.tile([128, 504], DT.float32, tag="ps")
                    nc.tensor.matmul(pszr[:], M["drt"][:], yt[:, sa], start=True, stop=False)
                    nc.tensor.matmul(pszr[:], M["ditn"][:], yt[:, sb], start=False, stop=True)
                    nc.tensor.matmul(pszi[:], M["drt"][:], yt[:, sb], start=True, stop=False)
                    nc.tensor.matmul(pszi[:], M["dit"][:], yt[:, sa], start=False, stop=True)
                    nc.vector.tensor_copy(zr[:, sa], pszr[:])
                    nc.vector.tensor_copy(zi[:, sa], pszi[:])

                # ---- TB: [g,(f,c)] -> [c,(f,g)]; 8-f groups; evict ACT ----
                zrt = pbpool.tile([C, FG], DT.bfloat16, tag="pb")
                zit = pbpool.tile([C, FG], DT.bfloat16, tag="pb")
                for (src, dst) in [(zr, zrt), (zi, zit)]:
                    sv = src[:].rearrange("p (f c) -> p f c", c=C)
                    if noturns:
                        for k in range(6):
                            s = slice(1024 * k, min(1024 * (k + 1), FC))
                            nc.scalar.copy(dst[0:96, s], src[0:96, s])
                        nc.scalar.copy(dst[0:96, FC:FG], dst[0:96, 0:FG - FC])
                        continue
                    for f0 in range(0, 64, 8):
                        psB = pstpool.tile([C, 1024], DT.bfloat16, tag="pst")
                        for j in range(8):
                            nc.tensor.transpose(
                                psB[:, 128 * j:128 * (j + 1)], sv[:, f0 + j, :], ID[:]
                            )
                        nc.scalar.copy(dst[:, 128 * f0:128 * (f0 + 8)], psB[:])
                    psB = pstpool.tile([C, 128], DT.bfloat16, tag="pst")
                    nc.tensor.transpose(psB[:], sv[:, 64, :], ID[:])
                    nc.scalar.copy(dst[:, 128 * 64:128 * 65], psB[:])

                # ---- L1 + L2 MLP fused; softshrink via two ReLUs ----
                o2r = pbpool.tile([C, FG], DT.bfloat16, tag="pb")
                o2i = pbpool.tile([C, FG], DT.bfloat16, tag="pb")
                chunks = [slice(512 * k, min(512 * (k + 1), FG))
                          for k in range((FG + 511) // 512)]
                for s in chunks:
                    n = s.stop - s.start
                    psr = pspool.tile([C, n], DT.float32, tag="ps")
                    psi = pspool.tile([C, n], DT.float32, tag="ps")
                    nc.tensor.matmul(psr[:], Wt["w1r"][:], zrt[:, s], start=True, stop=False)
                    nc.tensor.matmul(psr[:], Wt["w1in"][:], zit[:, s], start=False, stop=True)
                    nc.tensor.matmul(psi[:], Wt["w1r"][:], zit[:, s], start=True, stop=False)
                    nc.tensor.matmul(psi[:], Wt["w1i"][:], zrt[:, s], start=False, stop=True)
                    o1rc = spool.tile([C, 512], DT.bfloat16, tag="o1r", name="o1rc")
                    o1ic = spool.tile([C, 512], DT.bfloat16, tag="o1i", name="o1ic")
                    nc.scalar.activation(o1rc[:, :n], psr[:],
                                         mybir.ActivationFunctionType.Relu,
                                         bias=Bt["b1r"][:, 0:1])
                    nc.scalar.activation(o1ic[:, :n], psi[:],
                                         mybir.ActivationFunctionType.Relu,
                                         bias=Bt["b1i"][:, 0:1])
                    psr2 = pspool.tile([C, n], DT.float32, tag="ps")
                    psi2 = pspool.tile([C, n], DT.float32, tag="ps")
                    nc.tensor.matmul(psr2[:], Wt["w2r"][:], o1rc[:, :n], start=True, stop=False)
                    nc.tensor.matmul(psr2[:], Wt["w2in"][:], o1ic[:, :n], start=False, stop=True)
                    nc.tensor.matmul(psi2[:], Wt["w2r"][:], o1ic[:, :n], start=True, stop=False)
                    nc.tensor.matmul(psi2[:], Wt["w2i"][:], o1rc[:, :n], start=False, stop=True)
                    # softshrink(u) with u = psum + b2:
                    #   a1 = relu(psum + (b2 - lam)); a2 = relu(-psum + (-b2 - lam))
                    #   out = a1 - a2
                    for ps, bm, bp, dst in [(psr2, "b2rm", "b2rp", o2r),
                                            (psi2, "b2im", "b2ip", o2i)]:
                        a1 = spool.tile([C, 512], DT.bfloat16, tag="ss1", name="ssa1")
                        a2 = spool.tile([C, 512], DT.bfloat16, tag="ss2", name="ssa2")
                        nc.scalar.activation(a1[:, :n], ps[:],
                                             mybir.ActivationFunctionType.Relu,
                                             bias=Bt[bm][:, 0:1])
                        nc.scalar.activation(a2[:, :n], ps[:],
                                             mybir.ActivationFunctionType.Relu,
                                             bias=Bt[bp][:, 0:1], scale=-1.0)
                        nc.vector.tensor_sub(dst[:, s], a1[:, :n], a2[:, :n])

                # ---- TC: [c,(f,g)] -> [g,(f,c)]; 10-f groups; evict ACT ----
                or_ = papool.tile([128, FC], DT.bfloat16, tag="pa")
                oi_ = papool.tile([128, FC], DT.bfloat16, tag="pa")
                for (src, dst) in [(o2r, or_), (o2i, oi_)]:
                    sv = src[:].rearrange("p (f g) -> p f g", g=128)
                    if noturns:
                        for k in range(6):
                            s = slice(1024 * k, min(1024 * (k + 1), FC))
                            nc.scalar.copy(dst[0:96, s], src[0:96, s])
                        continue
                    for k in range(7):
                        f0 = 10 * k
                        nf = min(10, F - f0)
                        psC = pstpool.tile([128, 960], DT.bfloat16, tag="pst")
                        for j in range(nf):
                            nc.tensor.transpose(
                                psC[:, 96 * j:96 * (j + 1)], sv[:, f0 + j, :],
                                ID[0:96, 0:96]
                            )
                        nc.scalar.copy(dst[:, 96 * f0:96 * (f0 + nf)],
                                       psC[:, :96 * nf])

                # ---- S5: inverse H FFT; evicts DVE ----
                zb = bpool.tile([128, CH], DT.bfloat16, tag="big")
                zbv = zb[:].rearrange("p (c f) -> p f c", f=128)
                for j in range(13):
                    s = slice(480 * j, 480 * (j + 1))
                    pszr = pspool.tile([128, 480], DT.float32, tag="ps")
                    pszi = pspool.tile([128, 480], DT.float32, tag="ps")
                    nc.tensor.matmul(pszr[:], M["dirt"][:], or_[:, s], start=True, stop=False)
                    nc.tensor.matmul(pszr[:], M["diitn"][:], oi_[:, s], start=False, stop=True)
                    nc.tensor.matmul(pszi[:], M["dirt"][:], oi_[:, s], start=True, stop=False)
                    nc.tensor.matmul(pszi[:], M["diit"][:], or_[:, s], start=False, stop=True)
                    f0 = 5 * j
                    nc.vector.tensor_copy(zbv[:, f0:f0 + 5, :], pszr[:])
                    if j == 0:
                        nc.vector.tensor_copy(zbv[:, 65:69, :], pszi[:, 96:480])
                    elif j == 12:
                        nc.vector.tensor_copy(zbv[:, 124:128, :], pszi[:, 0:384])
                    else:
                        nc.vector.tensor_copy(zbv[:, 64 + f0:69 + f0, :], pszi[:])

                # ---- TD: 8-c groups; evict ACT ----
                zp = bpool.tile([128, CH], DT.bfloat16, tag="big")
                zbc = zb[:].rearrange("p (c f) -> p c f", f=128)
                zpv = zp[:].rearrange("p (h c) -> p c h", c=C)
                if noturns:
                    for k in range(CH // 1024):
                        s = slice(1024 * k, 1024 * (k + 1))
                        nc.vector.tensor_copy(zp[:, s], zb[:, s])
                else:
                    for c0 in range(0, C, 8):
                        psD = pstpool.tile([128, 1024], DT.bfloat16, tag="pst")
                        for j in range(8):
                            nc.tensor.transpose(
                                psD[:, 128 * j:128 * (j + 1)], zbc[:, c0 + j, :], ID[:]
                            )
                        nc.scalar.copy(zpv[:, c0:c0 + 8, :], psD[:])

                # ---- S6: inverse W rfft + f32 residual ----
                xrv = xres[b].rearrange("h w c -> w h c")
                orv = out_ext[b].rearrange("h w c -> w h c")
                for j in range(26):
                    lo = 480 * j
                    hi = min(lo + 480, CH)
                    n = hi - lo
                    h0, h1 = lo // C, hi // C
                    ps6 = pspool.tile([128, n], DT.float32, tag="ps")
                    nc.tensor.matmul(ps6[:], M["apack"][:], zp[:, lo:hi],
                                     start=True, stop=True)
                    oc = iopool.tile([128, 480], DT.float32, tag="oc")
                    xr = iopool.tile([128, 480], DT.float32, tag="xr")
                    nc.sync.dma_start(
                        xr[:, :n].rearrange("w (h c) -> w h c", c=C),
                        xrv[:, h0:h1, :])
                    nc.vector.tensor_add(oc[:, :n], ps6[:], xr[:, :n])
                    nc.sync.dma_start(orv[:, h0:h1, :],
                                      oc[:, :n].rearrange("w (h c) -> w h c", c=C))

    nc.compile()
    return nc


def _build_nc_tiny(reps=1):
    """Dispatch-floor calibration kernel: same I/O signature, ~no work."""
    nc = bacc.Bacc(
        "TRN2", target_bir_lowering=False, debug=False, num_devices=N_CORES
    )

    def din(name, shape, dt):
        return nc.dram_tensor(name, shape, dt, kind="ExternalInput")

    x16 = din("x16", [B, W, C, H], DT.bfloat16)
    din("xres", [B, H, W, C], DT.float32)
    for k in ["fpack", "drt", "dit", "ditn", "dirt", "diit", "diitn",
              "apack", "ident"]:
        din(k, [128, 128], DT.bfloat16)
    for k in ["w1r", "w1i", "w1in", "w2r", "w2i", "w2in"]:
        din(k, [C, C], DT.bfloat16)
    for k in ["b1r", "b1i", "b2r", "b2i"]:
        din(k, [C, 1], DT.float32)
    out_ext = nc.dram_tensor("out", [B, H, W, C], DT.float32, kind="ExternalOutput")

    with tile.TileContext(nc) as tc:
        with tc.tile_pool(name="p", bufs=2) as pool:
            for _ in range(reps):
                t = pool.tile([128, 512], DT.bfloat16, name="t")
                nc.sync.dma_start(
                    t[:], x16[0].rearrange("w c h -> w (c h)")[:, 0:512])
    nc.compile()
    return nc


def _build_nc_v2(reps=1, mode="v2"):
    """v2: corner turns on the DMA xbar (2-byte transpose), S2 output padded
    to c=128 per f so TB tiles are [128,128], bf16 residual from xb."""
    nc = bacc.Bacc(
        "TRN2", target_bir_lowering=False, debug=False, num_devices=N_CORES
    )

    def din(name, shape, dt):
        return nc.dram_tensor(name, shape, dt, kind="ExternalInput")

    x16 = din("x16", [B, W, C, H], DT.bfloat16)
    din("xres", [B, H, W, C], DT.float32)  # unused in v2, kept for same in_maps
    mats = {
        k: din(k, [128, 128], DT.bfloat16)
        for k in ["fpack", "drt", "dit", "ditn", "dirt", "diit", "diitn",
                  "apack", "ident"]
    }
    wts = {k: din(k, [C, C], DT.bfloat16)
           for k in ["w1r", "w1i", "w1in", "w2r", "w2i", "w2in"]}
    bs = {k: din(k, [C, 1], DT.float32) for k in ["b1r", "b1i", "b2r", "b2i"]}
    out_ext = nc.dram_tensor("out", [B, H, W, C], DT.float32, kind="ExternalOutput")

    CH = C * H          # 12288
    FC = F * C          # 6240
    FG = F * 128        # 8320

    with tile.TileContext(nc) as tc:
        with (
            tc.tile_pool(name="const", bufs=1) as cpool,
            tc.tile_pool(name="big", bufs=3) as bpool,
            tc.tile_pool(name="pa", bufs=3) as papool,
            tc.tile_pool(name="pb", bufs=4) as pbpool,
            tc.tile_pool(name="sm", bufs=2) as spool,
            tc.tile_pool(name="io", bufs=2) as iopool,
            tc.tile_pool(name="ps", bufs=8, space="PSUM") as pspool,
        ):
            M = {}
            for k in mats:
                M[k] = cpool.tile([128, 128], DT.bfloat16, tag=f"m_{k}", name=f"m_{k}")
                nc.sync.dma_start(M[k][:], mats[k][:])
            Wt = {}
            for k in wts:
                Wt[k] = cpool.tile([C, C], DT.bfloat16, tag=f"w_{k}", name=f"wt_{k}")
                nc.sync.dma_start(Wt[k][:], wts[k][:])
            Bt = {}
            for k in bs:
                Bt[k] = cpool.tile([C, 1], DT.float32, tag=f"b_{k}", name=f"bt_{k}")
                nc.sync.dma_start(Bt[k][:], bs[k][:])

            # main f=1..63 span chunking for S2: f-aligned (5f=480 cols), last 144
            s2_chunks = [(1 + 5 * j, min(6 + 5 * j, 64)) for j in range(13)]

            for b in [bb for _ in range(reps) for bb in range(B)]:
                xb = bpool.tile([128, CH], DT.bfloat16, tag="big")
                nc.sync.dma_start(xb[:], x16[b].rearrange("w c h -> w (c h)"))

                # ---- S1: W-rfft packed -> yp [fpack, (c, h)] ----
                yp = bpool.tile([128, CH], DT.bfloat16, tag="big")
                for k in range(CH // 512):
                    s = slice(512 * k, 512 * (k + 1))
                    ps = pspool.tile([128, 512], DT.float32, tag="ps")
                    nc.tensor.matmul(ps[:], M["fpack"][:], xb[:, s],
                                     start=True, stop=True)
                    nc.scalar.copy(yp[:, s], ps[:])

                # ---- TA (xbar): -> yt [h, (c, fpack)] ----
                yt = bpool.tile([128, CH], DT.bfloat16, tag="big")
                ypv = yp[:].rearrange("p (c h) -> p c h", h=H)
                ytv = yt[:].rearrange("p (c f) -> p c f", f=128)
                for c in range(C):
                    nc.sync.dma_start(ytv[:, c, :], ypv[:, c, :], transpose=True)

                # ---- S2: H-axis complex FFT -> zrp/zip [g, (f, c128-pad)] ----
                zrp = pbpool.tile([128, 65 * 128], DT.bfloat16, tag="pb")
                zip_ = pbpool.tile([128, 65 * 128], DT.bfloat16, tag="pb")
                zrpv = zrp[:].rearrange("p (f c) -> p c f", c=128)
                zipv = zip_[:].rearrange("p (f c) -> p c f", c=128)
                for f_edge in (0, 64):
                    pe1 = pspool.tile([128, 96], DT.float32, tag="ps")
                    pe2 = pspool.tile([128, 96], DT.float32, tag="ps")
                    nc.tensor.matmul(pe1[:], M["drt"][:], ytv[:, :, f_edge],
                                     start=True, stop=True)
                    nc.tensor.matmul(pe2[:], M["dit"][:], ytv[:, :, f_edge],
                                     start=True, stop=True)
                    nc.scalar.copy(zrpv[:, 0:96, f_edge], pe1[:])
                    nc.scalar.copy(zipv[:, 0:96, f_edge], pe2[:])
                for f0, f1 in s2_chunks:
                    n = (f1 - f0) * 96
                    sa = ytv[:, :, f0:f1]              # fr cols, (c, f) order
                    sb = ytv[:, :, 64 + f0:64 + f1]    # fi cols
                    pszr = pspool.tile([128, n], DT.float32, tag="ps")
                    pszi = pspool.tile([128, n], DT.float32, tag="ps")
                    nc.tensor.matmul(pszr[:], M["drt"][:], sa, start=True, stop=False)
                    nc.tensor.matmul(pszr[:], M["ditn"][:], sb, start=False, stop=True)
                    nc.tensor.matmul(pszi[:], M["drt"][:], sb, start=True, stop=False)
                    nc.tensor.matmul(pszi[:], M["dit"][:], sa, start=False, stop=True)
                    nc.vector.tensor_copy(
                        zrpv[:, 0:96, f0:f1],
                        pszr[:].rearrange("p (c f) -> p c f", c=96))
                    nc.vector.tensor_copy(
                        zipv[:, 0:96, f0:f1],
                        pszi[:].rearrange("p (c f) -> p c f", c=96))

                # ---- TB (xbar): -> zrt/zit [c128-pad, (f, g)] ----
                zrt = pbpool.tile([128, FG], DT.bfloat16, tag="pb")
                zit = pbpool.tile([128, FG], DT.bfloat16, tag="pb")
                for (src, dst) in [(zrp, zrt), (zip_, zit)]:
                    sv = src[:].rearrange("p (f c) -> p f c", c=128)
                    for f in range(F):
                        nc.sync.dma_start(dst[:, 128 * f:128 * (f + 1)],
                                          sv[:, f, :], transpose=True)

                # ---- L1 + L2 MLP fused ----
                o2r = pbpool.tile([C, FG], DT.bfloat16, tag="pb")
                o2i = pbpool.tile([C, FG], DT.bfloat16, tag="pb")
                chunks = [slice(512 * k, min(512 * (k + 1), FG))
                          for k in range((FG + 511) // 512)]
                for s in chunks:
                    n = s.stop - s.start
                    psr = pspool.tile([C, n], DT.float32, tag="ps")
                    psi = pspool.tile([C, n], DT.float32, tag="ps")
                    nc.tensor.matmul(psr[:], Wt["w1r"][:], zrt[0:96, s], start=True, stop=False)
                    nc.tensor.matmul(psr[:], Wt["w1in"][:], zit[0:96, s], start=False, stop=True)
                    nc.tensor.matmul(psi[:], Wt["w1r"][:], zit[0:96, s], start=True, stop=False)
                    nc.tensor.matmul(psi[:], Wt["w1i"][:], zrt[0:96, s], start=False, stop=True)
                    o1rc = spool.tile([C, 512], DT.bfloat16, tag="o1r", name="o1rc")
                    o1ic = spool.tile([C, 512], DT.bfloat16, tag="o1i", name="o1ic")
                    nc.scalar.activation(o1rc[:, :n], psr[:],
                                         mybir.ActivationFunctionType.Relu,
                                         bias=Bt["b1r"][:, 0:1])
                    nc.scalar.activation(o1ic[:, :n], psi[:],
                                         mybir.ActivationFunctionType.Relu,
                                         bias=Bt["b1i"][:, 0:1])
                    psr2 = pspool.tile([C, n], DT.float32, tag="ps")
                    psi2 = pspool.tile([C, n], DT.float32, tag="ps")
                    nc.tensor.matmul(psr2[:], Wt["w2r"][:], o1rc[:, :n], start=True, stop=False)
                    nc.tensor.matmul(psr2[:], Wt["w2in"][:], o1ic[:, :n], start=False, stop=True)
                    nc.tensor.matmul(psi2[:], Wt["w2r"][:], o1ic[:, :n], start=True, stop=False)
                    nc.tensor.matmul(psi2[:], Wt["w2i"][:], o1rc[:, :n], start=False, stop=True)
                    for ps, bias, dst in [(psr2, "b2r", o2r), (psi2, "b2i", o2i)]:
                        t = spool.tile([C, 512], DT.bfloat16, tag="ss1", name="sst")
                        tcl = spool.tile([C, 512], DT.bfloat16, tag="ss2", name="sscl")
                        nc.scalar.activation(t[:, :n], ps[:],
                                             mybir.ActivationFunctionType.Identity,
                                             bias=Bt[bias][:, 0:1])
                        nc.vector.tensor_scalar(
                            tcl[:, :n], t[:, :n], -LAM, LAM,
                            mybir.AluOpType.max, mybir.AluOpType.min)
                        nc.vector.tensor_sub(dst[:, s], t[:, :n], tcl[:, :n])

                # ---- TC (xbar): [c, (f, g)] -> or_/oi_ [g, (f, c)] ----
                or_ = papool.tile([128, FC], DT.bfloat16, tag="pa")
                oi_ = papool.tile([128, FC], DT.bfloat16, tag="pa")
                for (src, dst) in [(o2r, or_), (o2i, oi_)]:
                    sv = src[:].rearrange("p (f g) -> p f g", g=128)
                    for f in range(F):
                        nc.sync.dma_start(dst[:, 96 * f:96 * (f + 1)],
                                          sv[:, f, :], transpose=True)

                # ---- S5: inverse H FFT, packed -> zb [h, (c, fpack)] ----
                zb = bpool.tile([128, CH], DT.bfloat16, tag="big")
                zbv = zb[:].rearrange("p (c f) -> p f c", f=128)
                for j in range(13):
                    s = slice(480 * j, 480 * (j + 1))
                    pszr = pspool.tile([128, 480], DT.float32, tag="ps")
                    pszi = pspool.tile([128, 480], DT.float32, tag="ps")
                    nc.tensor.matmul(pszr[:], M["dirt"][:], or_[:, s], start=True, stop=False)
                    nc.tensor.matmul(pszr[:], M["diitn"][:], oi_[:, s], start=False, stop=True)
                    nc.tensor.matmul(pszi[:], M["dirt"][:], oi_[:, s], start=True, stop=False)
                    nc.tensor.matmul(pszi[:], M["diit"][:], or_[:, s], start=False, stop=True)
                    f0 = 5 * j
                    nc.vector.tensor_copy(zbv[:, f0:f0 + 5, :], pszr[:])
                    if j == 0:
                        nc.vector.tensor_copy(zbv[:, 65:69, :], pszi[:, 96:480])
                    elif j == 12:
                        nc.vector.tensor_copy(zbv[:, 124:128, :], pszi[:, 0:384])
                    else:
                        nc.vector.tensor_copy(zbv[:, 64 + f0:69 + f0, :], pszi[:])

                # ---- TD (PE): -> zp [fpack, (h, c)] ----
                # (xbar can't write the strided per-c output this layout needs,
                #  and S6's store wants c-contiguous runs, so TD stays on PE)
                zp = bpool.tile([128, CH], DT.bfloat16, tag="big")
                zbc = zb[:].rearrange("p (c f) -> p c f", f=128)
                zpv = zp[:].rearrange("p (h c) -> p c h", c=C)
                for c0 in range(0, C, 8):
                    psD = pspool.tile([128, 1024], DT.bfloat16, tag="ps")
                    for j in range(8):
                        nc.tensor.transpose(
                            psD[:, 128 * j:128 * (j + 1)], zbc[:, c0 + j, :],
                            M["ident"][:]
                        )
                    nc.scalar.copy(zpv[:, c0:c0 + 8, :], psD[:])

                # ---- S6: inverse W rfft + bf16 residual ----
                orv = out_ext[b].rearrange("h w c -> w h c")
                xbv = xb[:].rearrange("p (c h) -> p h c", h=H)
                for j in range(26):
                    lo = 480 * j
                    hi = min(lo + 480, CH)
                    n = hi - lo
                    h0, h1 = lo // C, hi // C
                    ps6 = pspool.tile([128, n], DT.float32, tag="ps")
                    nc.tensor.matmul(ps6[:], M["apack"][:], zp[:, lo:hi],
                                     start=True, stop=True)
                    oc = iopool.tile([128, 480], DT.float32, tag="oc")
                    nc.vector.tensor_add(
                        oc[:, :n].rearrange("w (h c) -> w h c", c=C),
                        ps6[:].rearrange("w (h c) -> w h c", c=C),
                        xbv[:, h0:h1, :])
                    nc.sync.dma_start(orv[:, h0:h1, :],
                                      oc[:, :n].rearrange("w (h c) -> w h c", c=C))

    nc.compile()
    return nc


def _get_compiled(reps=1, mode="full"):
    key = f"nc{reps}_{mode}"
    if key not in _CACHE:
        if "mats" not in _CACHE:
            _CACHE["mats"] = _host_matrices()
        _CACHE[key] = _build_nc(reps, mode)
    return _CACHE[key], _CACHE["mats"]


def make_in_maps(x, w1, b1, w2, b2):
    mats = _get_compiled()[1]
    in_maps = []
    for i in range(N_CORES):
        sl = slice(C * i, C * (i + 1))
        xs = x[:, :, :, sl]
        m = dict(mats)
        m["x16"] = np.ascontiguousarray(
            xs.transpose(0, 2, 3, 1).astype(BF16))  # [B, w, c, h]
        m["xres"] = np.ascontiguousarray(xs)
        m["w1r"] = np.ascontiguousarray(w1[0, i].astype(BF16))
        m["w1i"] = np.ascontiguousarray(w1[1, i].astype(BF16))
        m["w1in"] = np.ascontiguousarray((-w1[1, i]).astype(BF16))
        m["w2r"] = np.ascontiguousarray(w2[0, i].astype(BF16))
        m["w2i"] = np.ascontiguousarray(w2[1, i].astype(BF16))
        m["w2in"] = np.ascontiguousarray((-w2[1, i]).astype(BF16))
        m["b1r"] = np.ascontiguousarray(b1[0, i].astype(np.float32)[:, None])
        m["b1i"] = np.ascontiguousarray(b1[1, i].astype(np.float32)[:, None])
        m["b2r"] = np.ascontiguousarray(b2[0, i].astype(np.float32)[:, None])
        m["b2i"] = np.ascontiguousarray(b2[1, i].astype(np.float32)[:, None])
        m["b2rm"] = np.ascontiguousarray((b2[0, i] - LAM).astype(np.float32)[:, None])
        m["b2rp"] = np.ascontiguousarray((-b2[0, i] - LAM).astype(np.float32)[:, None])
        m["b2im"] = np.ascontiguousarray((b2[1, i] - LAM).astype(np.float32)[:, None])
        m["b2ip"] = np.ascontiguousarray((-b2[1, i] - LAM).astype(np.float32)[:, None])
        in_maps.append(m)
    return in_maps


def run(x, w1, b1, w2, b2, trace=False):
    nc, _ = _get_compiled()
    in_maps = make_in_maps(x, w1, b1, w2, b2)
    res = run_bass_kernel_spmd(nc, in_maps, core_ids=list(range(N_CORES)),
                               trace=trace)
    out = np.empty((B, H, W, C * N_CORES), dtype=np.float32)
    for i in range(N_CORES):
        out[:, :, :, C * i:C * (i + 1)] = res.results[i]["out"]
    return out, res


def kernel(x, w1, b1, w2, b2):
    x = np.asarray(x, dtype=np.float32)
    out, _ = run(x, np.asarray(w1), np.asarray(b1), np.asarray(w2),
                 np.asarray(b2))
    return out
